# revision 12
# baseline (speedup 1.0000x reference)
"""Trainium2 Bass kernel for nn_DynamicConvLayer.

The reference module's output is `where(offset_mag > 0.01, out, out)` == out,
i.e. exactly the main 3x3 conv (stride 1, pad 1) + bias. The offset branch is
dead code, so only the main conv is computed.

Strategy: pure data parallel over batch (16 images / 8 cores = 2 images per
core). Per image, the conv is 9 shifted matmuls over Cin=128 (partition dim)
accumulating into PSUM per 512-pixel (4 output rows) tile. float32r (TF32-like)
matmul dtype gives full PE throughput at ~1e-4 relative error.
"""
import sys

sys.path.insert(0, "/opt/trn_rl_repo")

import numpy as np

B, C, H, W = 16, 128, 128, 128
KK = 3
N_CORES = 8
IMGS_PER_CORE = B // N_CORES  # 2
PH, PW = H + 2, H + 4  # padded image in SBUF; PW=132 keeps each row 16B-aligned
# (cols 130/131 are never read: taps use cols kw..kw+127, kw<=2)
ROWS_PER_BLK = 4  # 4*W = 512 = one PSUM bank of fp32
N_BLKS = H // ROWS_PER_BLK  # 32
DMA_SLAB_ROWS = 16  # input rows per DMA (1 MiB per slab)
OUT_BATCH = 4  # row-blocks per output stage tile / DMA (1 MiB per out-DMA)
EVICT_ENGINE = "vector"  # which engine drains PSUM: vector | scalar | split
SKIP_IN_DMA = False   # ablation: drop input slab DMAs
SKIP_OUT_DMA = False  # ablation: drop output DMAs
PSUM_GROUP = 1  # row-blocks (banks) per PSUM tile; one eviction reads the whole tile

_compiled = None


def _build(reps=None):
    """Build the conv program. reps=N wraps the whole body in a For_i loop
    executing it N times (identical work; used only for differential timing —
    the production path uses reps=None → straight-line)."""
    from concourse import bacc, tile
    import concourse.mybir as mybir
    from contextlib import nullcontext

    f32 = mybir.dt.float32
    f32r = mybir.dt.float32r

    nc = bacc.Bacc("TRN2", target_bir_lowering=False, debug=False)

    x_d = nc.declare_dram_parameter("x", [IMGS_PER_CORE, C, H, W], f32r, isOutput=False)
    wt_d = nc.declare_dram_parameter("wt", [C, KK * KK, C], f32r, isOutput=False)
    bias_d = nc.declare_dram_parameter("bias", [C, 1], f32, isOutput=False)
    y_d = nc.declare_dram_parameter("y", [IMGS_PER_CORE, C, H * W], f32, isOutput=True)

    with tile.TileContext(nc) as tc:
        with (
            tc.tile_pool(name="imgs", bufs=1) as imgpool,
            tc.tile_pool(name="consts", bufs=1) as constpool,
            tc.tile_pool(name="stage", bufs=4) as stagepool,
            tc.tile_pool(name="psum", bufs=8 // PSUM_GROUP, space="PSUM") as psumpool,
        ):
            # consts ride the ACT HWDGE ring so the SP ring's first job is
            # image slab 0 (weights load in parallel with it)
            wt_sb = constpool.tile([C, KK * KK, C], f32r, tag="wt")
            nc.scalar.dma_start(wt_sb[:], wt_d[:])
            bias_sb = constpool.tile([C, 1], f32, tag="bias")
            nc.scalar.dma_start(bias_sb[:], bias_d[:])

            loop_cm = tc.For_i(0, reps, 1) if reps is not None else nullcontext()
            with loop_cm:
                _conv_body(nc, tc, imgpool, stagepool, psumpool, wt_sb, bias_sb, x_d, y_d, f32, f32r)

    nc.compile()
    return nc


def _conv_body(nc, tc, imgpool, stagepool, psumpool, wt_sb, bias_sb, x_d, y_d, f32, f32r):
    import concourse.mybir as mybir
    if True:
            imgs = []
            for b in range(IMGS_PER_CORE):
                img = imgpool.tile([C, PH, PW], f32r, tag=f"img{b}")
                # zero the halo ring; interior is fully overwritten by DMA
                # (memset doesn't take f32r APs — same bits as f32 zero)
                nc.vector.memset(img[:, 0, :].bitcast(f32), 0.0)
                nc.vector.memset(img[:, PH - 1, :].bitcast(f32), 0.0)
                nc.vector.memset(img[:, 1 : PH - 1, 0].bitcast(f32), 0.0)
                nc.vector.memset(img[:, 1 : PH - 1, PW - 1].bitcast(f32), 0.0)
                # small leading slabs let the first row-blocks start early
                slabs = [6, 10] + [DMA_SLAB_ROWS] * ((H - 16) // DMA_SLAB_ROWS) if b == 0 else [DMA_SLAB_ROWS] * (H // DMA_SLAB_ROWS)
                s = 0
                for rows in slabs:
                    if not SKIP_IN_DMA:
                        nc.sync.dma_start(
                            img[:, 1 + s : 1 + s + rows, 1 : 1 + W],
                            x_d[b, :, s : s + rows, :],
                        )
                    s += rows
                imgs.append(img)

            for b in range(IMGS_PER_CORE):
                img = imgs[b]
                for jg in range(N_BLKS // OUT_BATCH):
                    # one stage tile collects OUT_BATCH row-blocks -> one 1MiB DMA
                    stage = stagepool.tile([C, OUT_BATCH, ROWS_PER_BLK * W], f32)
                    for qg in range(OUT_BATCH // PSUM_GROUP):
                        # one PSUM tile spans PSUM_GROUP banks; one matmul
                        # group fills each bank, one DVE op drains them all
                        acc = psumpool.tile([C, PSUM_GROUP, ROWS_PER_BLK * W], f32)
                        for g in range(PSUM_GROUP):
                            q = qg * PSUM_GROUP + g
                            j = jg * OUT_BATCH + q
                            r = j * ROWS_PER_BLK
                            for t in range(KK * KK):
                                kh, kw = divmod(t, KK)
                                nc.tensor.matmul(
                                    acc[:, g, :],
                                    wt_sb[:, t, :],
                                    img[:, r + kh : r + kh + ROWS_PER_BLK, kw : kw + W],
                                    start=(t == 0),
                                    stop=(t == KK * KK - 1),
                                )
                        nc.vector.tensor_scalar_add(
                            stage[:, qg * PSUM_GROUP : (qg + 1) * PSUM_GROUP, :],
                            acc[:],
                            bias_sb[:],
                        )
                    rg = jg * OUT_BATCH * ROWS_PER_BLK
                    # ACT's HWDGE ring: keeps output DMAs (which wait on
                    # compute) off the SP ring that streams input slabs,
                    # avoiding head-of-line blocking there.
                    if not SKIP_OUT_DMA:
                        nc.scalar.dma_start(
                            y_d[b, :, rg * W : (rg + OUT_BATCH * ROWS_PER_BLK) * W],
                            stage[:],
                        )
                    elif b == IMGS_PER_CORE - 1 and jg == N_BLKS // OUT_BATCH - 1:
                        nc.scalar.dma_start(y_d[0, :, 0:512], stage[:, 0, :])


def kernel(**inputs: np.ndarray) -> np.ndarray:
    global _compiled
    from concourse.bass_utils import run_bass_kernel_spmd

    x = np.ascontiguousarray(inputs["x"], dtype=np.float32)
    main_w = np.asarray(inputs["main_w"], dtype=np.float32)
    main_b = np.asarray(inputs["main_b"], dtype=np.float32)

    # [Cout, Cin, kh, kw] -> [Cin, kh*kw, Cout] (lhsT per tap)
    wt = np.ascontiguousarray(main_w.transpose(1, 2, 3, 0).reshape(C, KK * KK, C))
    bias = np.ascontiguousarray(main_b.reshape(C, 1))

    if _compiled is None:
        _compiled = _build()

    core_ids = list(range(N_CORES))
    in_maps = [
        {
            "x": np.ascontiguousarray(x[c * IMGS_PER_CORE : (c + 1) * IMGS_PER_CORE]),
            "wt": wt,
            "bias": bias,
        }
        for c in core_ids
    ]
    res = run_bass_kernel_spmd(_compiled, in_maps, core_ids)
    y = np.concatenate([r["y"].reshape(IMGS_PER_CORE, C, H, W) for r in res.results])
    return y.astype(np.float32)


if __name__ == "__main__":
    rng = np.random.default_rng(0)
    inputs = {
        "x": rng.standard_normal((B, C, H, W), dtype=np.float32),
        "main_w": rng.standard_normal((C, C, KK, KK), dtype=np.float32) * 0.02,
        "main_b": rng.standard_normal((C,), dtype=np.float32) * 0.02,
    }
    y = kernel(**inputs)
    print(y.shape, y.dtype)


# revision 13
# speedup vs baseline: 1.0009x; 1.0009x over previous
"""Trainium2 Bass kernel for nn_DynamicConvLayer.

The reference module's output is `where(offset_mag > 0.01, out, out)` == out,
i.e. exactly the main 3x3 conv (stride 1, pad 1) + bias. The offset branch is
dead code, so only the main conv is computed.

Strategy: pure data parallel over batch (16 images / 8 cores = 2 images per
core). Per image, the conv is 9 shifted matmuls over Cin=128 (partition dim)
accumulating into PSUM per 512-pixel (4 output rows) tile. float32r (TF32-like)
matmul dtype gives full PE throughput at ~1e-4 relative error.
"""
import sys

sys.path.insert(0, "/opt/trn_rl_repo")

import numpy as np

B, C, H, W = 16, 128, 128, 128
KK = 3
N_CORES = 8
IMGS_PER_CORE = B // N_CORES  # 2
PH, PW = H + 2, H + 4  # padded image in SBUF; PW=132 keeps each row 16B-aligned
# (cols 130/131 are never read: taps use cols kw..kw+127, kw<=2)
ROWS_PER_BLK = 4  # 4*W = 512 = one PSUM bank of fp32
N_BLKS = H // ROWS_PER_BLK  # 32
DMA_SLAB_ROWS = 16  # input rows per DMA (1 MiB per slab)
OUT_BATCH = 2  # row-blocks per output stage tile / DMA (1 MiB per out-DMA)
EVICT_ENGINE = "vector"  # which engine drains PSUM: vector | scalar | split
SKIP_IN_DMA = False   # ablation: drop input slab DMAs
SKIP_OUT_DMA = False  # ablation: drop output DMAs
PSUM_GROUP = 1  # row-blocks (banks) per PSUM tile; one eviction reads the whole tile

_compiled = None


def _build(reps=None):
    """Build the conv program. reps=N wraps the whole body in a For_i loop
    executing it N times (identical work; used only for differential timing —
    the production path uses reps=None → straight-line)."""
    from concourse import bacc, tile
    import concourse.mybir as mybir
    from contextlib import nullcontext

    f32 = mybir.dt.float32
    f32r = mybir.dt.float32r

    nc = bacc.Bacc("TRN2", target_bir_lowering=False, debug=False)

    x_d = nc.declare_dram_parameter("x", [IMGS_PER_CORE, C, H, W], f32r, isOutput=False)
    wt_d = nc.declare_dram_parameter("wt", [C, KK * KK, C], f32r, isOutput=False)
    bias_d = nc.declare_dram_parameter("bias", [C, 1], f32, isOutput=False)
    y_d = nc.declare_dram_parameter("y", [IMGS_PER_CORE, C, H * W], f32, isOutput=True)

    with tile.TileContext(nc) as tc:
        with (
            tc.tile_pool(name="imgs", bufs=1) as imgpool,
            tc.tile_pool(name="consts", bufs=1) as constpool,
            tc.tile_pool(name="stage", bufs=4) as stagepool,
            tc.tile_pool(name="psum", bufs=8 // PSUM_GROUP, space="PSUM") as psumpool,
        ):
            # consts ride the ACT HWDGE ring so the SP ring's first job is
            # image slab 0 (weights load in parallel with it)
            wt_sb = constpool.tile([C, KK * KK, C], f32r, tag="wt")
            nc.scalar.dma_start(wt_sb[:], wt_d[:])
            bias_sb = constpool.tile([C, 1], f32, tag="bias")
            nc.scalar.dma_start(bias_sb[:], bias_d[:])

            loop_cm = tc.For_i(0, reps, 1) if reps is not None else nullcontext()
            with loop_cm:
                _conv_body(nc, tc, imgpool, stagepool, psumpool, wt_sb, bias_sb, x_d, y_d, f32, f32r)

    nc.compile()
    return nc


def _conv_body(nc, tc, imgpool, stagepool, psumpool, wt_sb, bias_sb, x_d, y_d, f32, f32r):
    import concourse.mybir as mybir
    if True:
            imgs = []
            for b in range(IMGS_PER_CORE):
                img = imgpool.tile([C, PH, PW], f32r, tag=f"img{b}")
                # zero the halo ring; interior is fully overwritten by DMA
                # (memset doesn't take f32r APs — same bits as f32 zero)
                nc.vector.memset(img[:, 0, :].bitcast(f32), 0.0)
                nc.vector.memset(img[:, PH - 1, :].bitcast(f32), 0.0)
                nc.vector.memset(img[:, 1 : PH - 1, 0].bitcast(f32), 0.0)
                nc.vector.memset(img[:, 1 : PH - 1, PW - 1].bitcast(f32), 0.0)
                # small leading slabs let the first row-blocks start early
                slabs = [6, 10] + [DMA_SLAB_ROWS] * ((H - 16) // DMA_SLAB_ROWS) if b == 0 else [DMA_SLAB_ROWS] * (H // DMA_SLAB_ROWS)
                s = 0
                for rows in slabs:
                    if not SKIP_IN_DMA:
                        nc.sync.dma_start(
                            img[:, 1 + s : 1 + s + rows, 1 : 1 + W],
                            x_d[b, :, s : s + rows, :],
                        )
                    s += rows
                imgs.append(img)

            for b in range(IMGS_PER_CORE):
                img = imgs[b]
                for jg in range(N_BLKS // OUT_BATCH):
                    # one stage tile collects OUT_BATCH row-blocks -> one 1MiB DMA
                    stage = stagepool.tile([C, OUT_BATCH, ROWS_PER_BLK * W], f32)
                    for qg in range(OUT_BATCH // PSUM_GROUP):
                        # one PSUM tile spans PSUM_GROUP banks; one matmul
                        # group fills each bank, one DVE op drains them all
                        acc = psumpool.tile([C, PSUM_GROUP, ROWS_PER_BLK * W], f32)
                        for g in range(PSUM_GROUP):
                            q = qg * PSUM_GROUP + g
                            j = jg * OUT_BATCH + q
                            r = j * ROWS_PER_BLK
                            for t in range(KK * KK):
                                kh, kw = divmod(t, KK)
                                nc.tensor.matmul(
                                    acc[:, g, :],
                                    wt_sb[:, t, :],
                                    img[:, r + kh : r + kh + ROWS_PER_BLK, kw : kw + W],
                                    start=(t == 0),
                                    stop=(t == KK * KK - 1),
                                )
                        nc.vector.tensor_scalar_add(
                            stage[:, qg * PSUM_GROUP : (qg + 1) * PSUM_GROUP, :],
                            acc[:],
                            bias_sb[:],
                        )
                    rg = jg * OUT_BATCH * ROWS_PER_BLK
                    # ACT's HWDGE ring: keeps output DMAs (which wait on
                    # compute) off the SP ring that streams input slabs,
                    # avoiding head-of-line blocking there.
                    if not SKIP_OUT_DMA:
                        nc.scalar.dma_start(
                            y_d[b, :, rg * W : (rg + OUT_BATCH * ROWS_PER_BLK) * W],
                            stage[:],
                        )
                    elif b == IMGS_PER_CORE - 1 and jg == N_BLKS // OUT_BATCH - 1:
                        nc.scalar.dma_start(y_d[0, :, 0:512], stage[:, 0, :])


def kernel(**inputs: np.ndarray) -> np.ndarray:
    global _compiled
    from concourse.bass_utils import run_bass_kernel_spmd

    x = np.ascontiguousarray(inputs["x"], dtype=np.float32)
    main_w = np.asarray(inputs["main_w"], dtype=np.float32)
    main_b = np.asarray(inputs["main_b"], dtype=np.float32)

    # [Cout, Cin, kh, kw] -> [Cin, kh*kw, Cout] (lhsT per tap)
    wt = np.ascontiguousarray(main_w.transpose(1, 2, 3, 0).reshape(C, KK * KK, C))
    bias = np.ascontiguousarray(main_b.reshape(C, 1))

    if _compiled is None:
        _compiled = _build()

    core_ids = list(range(N_CORES))
    in_maps = [
        {
            "x": np.ascontiguousarray(x[c * IMGS_PER_CORE : (c + 1) * IMGS_PER_CORE]),
            "wt": wt,
            "bias": bias,
        }
        for c in core_ids
    ]
    res = run_bass_kernel_spmd(_compiled, in_maps, core_ids)
    y = np.concatenate([r["y"].reshape(IMGS_PER_CORE, C, H, W) for r in res.results])
    return y.astype(np.float32)


if __name__ == "__main__":
    rng = np.random.default_rng(0)
    inputs = {
        "x": rng.standard_normal((B, C, H, W), dtype=np.float32),
        "main_w": rng.standard_normal((C, C, KK, KK), dtype=np.float32) * 0.02,
        "main_b": rng.standard_normal((C,), dtype=np.float32) * 0.02,
    }
    y = kernel(**inputs)
    print(y.shape, y.dtype)
